# revision 51
# baseline (speedup 1.0000x reference)
"""CrossAttention Trainium2 kernel (Bass/Tile), batch-parallel over 8 NeuronCores.

Problem (per batch b of 8):
    x   [512, 32, 32]  -> X   [C=512, N=1024]
    ctx [512, 32, 32]  -> CTX [C=512, M=1024]
    q = Wq@X * s + bq*s ; k = Wk@CTX + bk ; v = Wv@CTX + bv     (1x1 convs)
    per head h (8 heads x 64): simT[j,i] = sum_d k[d,j] q[d,i]
    attn = softmax_j(sim);  out[i,d] = sum_j attn[i,j] v[d,j]
    final = Wo@out + bo

Layout strategy (per core = one batch):
  - channels live on partitions in chunks of 128 (4 chunks = head pairs);
    tokens on the free axis
  - sim is computed TRANSPOSED (j on partitions) so the attn@v contraction has
    j on partitions (PE contracts partitions)
  - attn@v runs with the exp'd sim BLOCK as the STATIONARY operand and v as the
    moving operand: out tile is [i=128 part, d=64 free] (half the moving rows
    of the [d, i-512] orientation). Softmax denominators come from 1-wide
    matmuls against a ones vector (free size 1 ~ free on the PE cost model)
  - normalization (x 1/den) happens per-partition on DVE (denominator lives on
    the same partition as the outputs it scales - no partition broadcast)
  - the normalized attn output [i, c] is flipped to [c, i] for the output
    projection by DMA-XBAR transposes (idle DMA engines, not PE)
  - sim/exp runs as 8 per-jc steps double-buffered across two 2-bank psum
    pools; projections stream through a separate 2-buffer job pool so their
    drains never stall the sim pipeline
  - wk + biases ride the Pool SWDGE queue and x/ctx/wq interleave across the
    two HWDGE queues so the q/k boot's critical tensors land first and the
    ACT sequencer stays clear for the exp stream
  - the LAST head pair's attn@v runs in the [d, i] orientation against a
    [v | 1] stationary so its normalized output lands directly in oallT
    (no normalize->transpose chain after the final exp)

Host-side prep (NOT device time): weights are pre-transposed and pre-cast to
bf16, the 1/sqrt(dim_head) scale is folded into Wq/bq.
"""

import contextlib
import os
import sys

sys.path.insert(0, "/opt/trn_rl_repo")

import numpy as np
import ml_dtypes

import concourse.bass as bass
import concourse.tile as tile
from concourse import bacc, mybir

B = 8
HEADS = 8
DH = 64
C = 512
NTOK = 1024  # 32*32
P = 128
CCH = C // P  # 4 channel chunks (= head pairs)
JCH = NTOK // P  # 8 context-token chunks (partition dim of simT)
ICH = 2  # query-token chunks of 512 (free dim)
SUB = 4  # 128-token sub-chunks per ic
F = 512
SCALE = DH ** (-0.5)

BF16 = mybir.dt.bfloat16
F32 = mybir.dt.float32
NPBF16 = ml_dtypes.bfloat16


def build_nc(reps: int = 1):
    nc = bacc.Bacc("TRN2", target_bir_lowering=False, debug=False)

    x_d = nc.dram_tensor("x", [C, NTOK], BF16, kind="ExternalInput")
    c_d = nc.dram_tensor("ctx", [C, NTOK], BF16, kind="ExternalInput")
    wqt_d = nc.dram_tensor("wqt", [C, C], BF16, kind="ExternalInput")
    wkt_d = nc.dram_tensor("wkt", [C, C], BF16, kind="ExternalInput")
    wvt_d = nc.dram_tensor("wvt", [C, C], BF16, kind="ExternalInput")
    wot_d = nc.dram_tensor("wot", [C, C], BF16, kind="ExternalInput")
    bq_d = nc.dram_tensor("bq", [C], F32, kind="ExternalInput")
    bk_d = nc.dram_tensor("bk", [C], F32, kind="ExternalInput")
    bv_d = nc.dram_tensor("bv", [C], F32, kind="ExternalInput")
    bo_d = nc.dram_tensor("bo", [C], F32, kind="ExternalInput")
    out_d = nc.dram_tensor("out", [C, NTOK], F32, kind="ExternalOutput")

    with tile.TileContext(nc) as tc:
        with (
            tc.tile_pool(name="consts", bufs=1) as consts,
            tc.tile_pool(name="acts", bufs=1) as acts,
            tc.tile_pool(name="expp", bufs=4) as expp,
            tc.tile_pool(name="sbcp", bufs=4) as sbcp,
            tc.tile_pool(name="normp", bufs=1) as normp,
            tc.tile_pool(name="finp", bufs=4) as finp,
            tc.tile_pool(name="simA", bufs=1, space="PSUM") as simA,
            tc.tile_pool(name="simB", bufs=1, space="PSUM") as simB,
            tc.tile_pool(name="attp", bufs=1, space="PSUM") as attp,
            tc.tile_pool(name="mxp", bufs=1, space="PSUM") as mxp,
            tc.tile_pool(name="jp", bufs=2, space="PSUM") as jp,
        ):
          with (tc.For_i(0, reps, 1) if reps > 1 else contextlib.nullcontext()) as _i:
            # ---- constants ----
            wq_sb = consts.tile([P, CCH, C], BF16, tag="wq")
            wk_sb = consts.tile([P, CCH, C], BF16, tag="wk")
            wv_sb = consts.tile([P, CCH, C], BF16, tag="wv")
            wo_sb = consts.tile([P, CCH, C], BF16, tag="wo")

            bq_sb = consts.tile([P, CCH], F32, tag="bq")
            bk_sb = consts.tile([P, CCH], F32, tag="bk")
            bo_sb = consts.tile([P, CCH], F32, tag="bo")
            ones_sb = consts.tile([P, 1], BF16, tag="ones")
            nc.vector.memset(ones_sb[:, :], 1.0)
            # bv broadcast across partitions: [128, 512] (free axis = channel)
            bv_bc = consts.tile([P, C], F32, tag="bvbc")
            b_ap = bv_d[None, :]
            bv_src = bass.AP(
                tensor=b_ap.tensor, offset=b_ap.offset, ap=[[0, P]] + list(b_ap.ap[1:])
            )

            # ---- loads. x and ctx stream on separate HWDGE queues in
            # parallel (x+wq gate the q boot, ctx+wk the k boot); wk/wo and
            # the biases ride the Pool SWDGE queue so the two HWDGE queues
            # stay short and the ACT sequencer frees up early.
            x_sb = acts.tile([P, CCH, NTOK], BF16, tag="x")
            c_sb = acts.tile([P, CCH, NTOK], BF16, tag="c")
            for b_sb, b_d in ((bq_sb, bq_d), (bk_sb, bk_d)):
                nc.gpsimd.dma_start(out=b_sb[:, :], in_=b_d.rearrange("(a p) -> p a", p=P))
            for cc in range(CCH):
                nc.gpsimd.dma_start(out=wk_sb[:, cc, :], in_=wkt_d[cc * P : (cc + 1) * P, :])
            for cc in range(2):
                nc.sync.dma_start(out=x_sb[:, cc, :], in_=x_d[cc * P : (cc + 1) * P, :])
                nc.sync.dma_start(out=c_sb[:, cc, :], in_=c_d[cc * P : (cc + 1) * P, :])
                nc.scalar.dma_start(out=wq_sb[:, cc, :], in_=wqt_d[cc * P : (cc + 1) * P, :])
                nc.scalar.dma_start(
                    out=c_sb[:, cc + 2, :], in_=c_d[(cc + 2) * P : (cc + 3) * P, :]
                )
            for cc in range(2, CCH):
                nc.sync.dma_start(out=x_sb[:, cc, :], in_=x_d[cc * P : (cc + 1) * P, :])
                nc.scalar.dma_start(out=wq_sb[:, cc, :], in_=wqt_d[cc * P : (cc + 1) * P, :])
            nc.gpsimd.dma_start(out=bv_bc[:, :], in_=bv_src)
            for cc in range(CCH):
                nc.sync.dma_start(out=wv_sb[:, cc, :], in_=wvt_d[cc * P : (cc + 1) * P, :])
            for cc in range(CCH):
                nc.sync.dma_start(out=wo_sb[:, cc, :], in_=wot_d[cc * P : (cc + 1) * P, :])
            nc.gpsimd.dma_start(out=bo_sb[:, :], in_=bo_d.rearrange("(a p) -> p a", p=P))

            q_sb = acts.tile([P, CCH, NTOK], BF16, tag="q")
            k_sb = acts.tile([P, CCH, NTOK], BF16, tag="k")
            # vT per head: [j-part, j-chunk, head, 64]
            vt_sb = acts.tile([P, JCH, HEADS, DH], BF16, tag="vt")
            # normalized attention output, [i-part, sub, c] per ic
            norm_sb = [
                normp.tile([P, SUB, C], BF16, tag=f"norm{ic}", name=f"norm{ic}")
                for ic in range(ICH)
            ]
            # transposed attention output for o-proj: [c-part, cc, i] per ic
            oallT = [
                acts.tile([P, CCH, F], BF16, tag=f"oallT{ic}", name=f"oallT{ic}")
                for ic in range(ICH)
            ]

            # the last head pair (6,7) also keeps a [v | 1] copy: the final
            # pair's attn@v runs in the [d, i] orientation whose output drops
            # straight into oallT (no normalize->transpose in the tail), with
            # the ones column supplying the softmax denominators
            vt1_sb = acts.tile([P, JCH, 2, DH + 1], BF16, tag="vt1")
            nc.vector.memset(vt1_sb[:, :, :, DH : DH + 1], 1.0)

            def emit_v_group(mc):
                ps = jp.tile([P, F], F32, tag="jp", name=f"vps{mc}")
                for cc in range(CCH):
                    nc.tensor.matmul(
                        ps[:, :],
                        c_sb[:, cc, mc * P : (mc + 1) * P],
                        wv_sb[:, cc, :],
                        start=(cc == 0),
                        stop=(cc == CCH - 1),
                    )
                nc.vector.tensor_tensor(
                    vt_sb[:, mc, :, :],
                    ps.rearrange("p (h d) -> p h d", d=DH),
                    bv_bc.rearrange("p (h d) -> p h d", d=DH),
                    mybir.AluOpType.add,
                )
                nc.vector.tensor_tensor(
                    vt1_sb[:, mc, :, 0:DH],
                    ps.rearrange("p (h d) -> p h d", d=DH)[:, 6:8, :],
                    bv_bc.rearrange("p (h d) -> p h d", d=DH)[:, 6:8, :],
                    mybir.AluOpType.add,
                )

            def emit_qk_group(which, oc, ih, drain=None):
                dst, wt, bias_t, src_sb = (
                    (q_sb, wq_sb, bq_sb, x_sb) if which == "q" else (k_sb, wk_sb, bk_sb, c_sb)
                )
                ps = jp.tile([P, F], F32, tag="jp", name=f"{which}ps{oc}{ih}")
                for cc in range(CCH):
                    nc.tensor.matmul(
                        ps[:, :],
                        wt[:, cc, oc * P : (oc + 1) * P],
                        src_sb[:, cc, ih * F : (ih + 1) * F],
                        start=(cc == 0),
                        stop=(cc == CCH - 1),
                    )
                (drain or nc.vector).tensor_tensor(
                    dst[:, oc, ih * F : (ih + 1) * F],
                    ps[:, :],
                    bias_t[:, oc : oc + 1].to_broadcast([P, F]),
                    mybir.AluOpType.add,
                )

            # boot: only what sim(0,0) needs early - q pr0 (first token half)
            # and k pr0 (first half; second half is the first loop job). The
            # q drain runs on the idle gpsimd engine so the two boot drains
            # don't serialize on DVE in front of the first sim step. ih=1
            # q-halves defer to iteration (0,3); v + the rest of q/k stream
            # inside the attention loop
            emit_qk_group("q", 0, 0)
            emit_qk_group("k", 0, 0)

            # ---- transposes + o-projection helpers ----
            def emit_transpose(ic, sub):
                # norm [i, c] -> oallT [c-part, cc, i] ; one xbar DMA per sub
                nc.sync.dma_start_transpose(
                    oallT[ic][:, :, sub * P : (sub + 1) * P],
                    norm_sb[ic][:, sub, :],
                )

            # oproj runs per 128-token sub-chunk; fins for a sub PAIR land in
            # one tile and ship as one DMA (halves the HWDGE store count)
            fin_tiles = {}

            def emit_oproj_sub(ic, sub, pool, tag):
                # ic0: fins for a sub PAIR land in one tile and ship as one
                # DMA (fewer HWDGE ops, hidden mid-flight). ic1 (the tail):
                # per-sub stores alternating across both HWDGE queues so the
                # final transfers pipeline instead of serializing.
                ii = SUB * ic + sub
                ps = pool.tile([P, CCH, P], F32, tag=tag, name=f"ops{ic}{sub}")
                for oc in range(CCH):
                    for cc in range(CCH):
                        nc.tensor.matmul(
                            ps[:, oc, :],
                            wo_sb[:, cc, oc * P : (oc + 1) * P],
                            oallT[ic][:, cc, sub * P : (sub + 1) * P],
                            start=(oc == 0 and cc == 0),
                            stop=(oc == CCH - 1 and cc == CCH - 1),
                        )
                out_view = out_d.rearrange("(o p) n -> p o n", p=P)
                if ic == 0:
                    pair = ii // 2
                    if pair not in fin_tiles:
                        fin_tiles[pair] = finp.tile(
                            [P, CCH, 2, P], F32, tag="fin", name=f"fin{pair}"
                        )
                    fin = fin_tiles[pair]
                    nc.vector.tensor_tensor(
                        fin[:, :, ii % 2, :],
                        ps[:, :, :],
                        bo_sb[:, :].to_broadcast([P, CCH, P]),
                        mybir.AluOpType.add,
                    )
                    if ii % 2 == 1:
                        nc.sync.dma_start(
                            out=out_view[:, :, pair * 2 * P : (pair + 1) * 2 * P],
                            in_=fin[:, :, :, :],
                        )
                else:
                    fin = finp.tile([P, CCH, 2, P], F32, tag="fin", name=f"finT{sub}")
                    nc.vector.tensor_tensor(
                        fin[:, :, 0, :],
                        ps[:, :, :],
                        bo_sb[:, :].to_broadcast([P, CCH, P]),
                        mybir.AluOpType.add,
                    )
                    q = nc.sync if sub % 2 == 0 else nc.scalar
                    q.dma_start(
                        out=out_view[:, :, ii * P : (ii + 1) * P],
                        in_=fin[:, :, 0, :],
                    )

            # ---- attention (software-pipelined, proj-merged) ----
            # 8 per-jc sim steps alternate between the two 2-bank psum pools;
            # the previous pair's attn@v matmuls (stationary = exp'd sim block,
            # moving = v) weave between sim steps as PE filler.

            # att/den tiles are ONE psum bank each with 8 interleaved
            # accumulation chains: matmul start zeroes the whole 2KB zero
            # region, so exactly one start (first matmul into the bank) and one
            # stop (last) per tile - intermediate writes land on pending-zero
            # bytes and accumulate from there.
            def emit_den_chunk(pes, den, jcs):
                # 1-wide denominator matmuls: out[i, 0] = sum_j exp[j, i]
                for jc in jcs:
                    for sub in range(SUB):
                        for hb in range(2):
                            nc.tensor.matmul(
                                den[:, sub, hb : hb + 1],
                                pes[:, jc, hb, sub * P : (sub + 1) * P],
                                ones_sb[:, :],
                                start=(jc == 0 and sub == 0 and hb == 0),
                                stop=(jc == JCH - 1 and sub == SUB - 1 and hb == 1),
                            )

            def emit_att_chunk(pes, ppr, att, jcs):
                for jc in jcs:
                    for sub in range(SUB):
                        for hb in range(2):
                            nc.tensor.matmul(
                                att[:, sub, hb, :],
                                pes[:, jc, hb, sub * P : (sub + 1) * P],
                                vt_sb[:, jc, 2 * ppr + hb, :],
                                start=(jc == 0 and sub == 0 and hb == 0),
                                stop=(jc == JCH - 1 and sub == SUB - 1 and hb == 1),
                            )

            def emit_normalize(pic, ppr, att, den_sb, subs=None):
                if subs is None:
                    nc.vector.tensor_tensor(
                        norm_sb[pic][:, :, ppr * P : (ppr + 1) * P].rearrange(
                            "p s (h d) -> p s h d", d=DH
                        ),
                        att[:, :, :, :],
                        den_sb[:, :, :].to_broadcast([P, SUB, 2, DH]),
                        mybir.AluOpType.mult,
                    )
                    return
                for sub in subs:
                    nc.vector.tensor_tensor(
                        norm_sb[pic][:, sub, ppr * P : (ppr + 1) * P].rearrange(
                            "p (h d) -> p h d", d=DH
                        ),
                        att[:, sub, :, :],
                        den_sb[:, sub, :].to_broadcast([P, 2, DH]),
                        mybir.AluOpType.mult,
                    )

            iters = [(ic, pr) for ic in range(ICH) for pr in range(CCH)]
            prev = None
            satt = sden = None
            for it_idx, (ic, pr) in enumerate(iters):
                last = it_idx == len(iters) - 1
                es = expp.tile([P, JCH, 2, F], BF16, tag="es", name=f"es{ic}{pr}")

                # proj jobs for this iteration, spread across the steps.
                # oproj jobs go late (they block on the ic0 transposes)
                proj_jobs = []
                job_first_slot = 1
                if (ic, pr) == (0, 0):
                    proj_jobs.append(lambda: emit_qk_group("k", 0, 1))
                    for mc in range(4):
                        proj_jobs.append(lambda m=mc: emit_v_group(m))
                    proj_jobs.append(lambda: emit_qk_group("q", 1, 0))
                    for ih in range(ICH):
                        proj_jobs.append(lambda i=ih: emit_qk_group("k", 1, i))
                elif (ic, pr) == (0, 1):
                    for mc in range(4, JCH):
                        proj_jobs.append(lambda m=mc: emit_v_group(m))
                    proj_jobs.append(lambda: emit_qk_group("q", 2, 0))
                    for ih in range(ICH):
                        proj_jobs.append(lambda i=ih: emit_qk_group("k", 2, i))
                elif (ic, pr) == (0, 2):
                    proj_jobs.append(lambda: emit_qk_group("q", 3, 0))
                    for ih in range(ICH):
                        proj_jobs.append(lambda i=ih: emit_qk_group("k", 3, i))
                elif (ic, pr) == (0, 3):
                    for oc in range(CCH):
                        proj_jobs.append(lambda o=oc: emit_qk_group("q", o, 1))
                elif (ic, pr) == (1, 1):
                    job_first_slot = 6
                    proj_jobs.append(lambda: emit_oproj_sub(0, 0, jp, "jp"))
                    proj_jobs.append(lambda: emit_oproj_sub(0, 1, jp, "jp"))
                elif (ic, pr) == (1, 2):
                    job_first_slot = 6
                    proj_jobs.append(lambda: emit_oproj_sub(0, 2, jp, "jp"))
                    proj_jobs.append(lambda: emit_oproj_sub(0, 3, jp, "jp"))

                att = den = den_sb = None
                if prev is not None:
                    pic, ppr, pes = prev
                    att = attp.tile([P, SUB, 2, DH], F32, tag="att", name=f"att{pic}{ppr}")
                    den = mxp.tile([P, SUB, 2], F32, tag="mx", name=f"den{pic}{ppr}")
                    den_sb = sbcp.tile([P, SUB, 2], F32, tag="den", name=f"dsb{pic}{ppr}")

                # Normal iteration: den burst at step 0, recip at step 1, att
                # chunks woven per step, normalize after the steps.
                # Last iteration: prev's chunks are front-loaded (steps 0-1) so
                # prev's normalize can run at step 2, freeing the att/mx psum
                # slots for this iteration's own self-chased attn@v.
                att_chunks = {}
                if prev is not None:
                    if not last:
                        att_chunks = {si: (si,) for si in range(JCH)}
                    else:
                        att_chunks = {0: (0, 1, 2, 3), 1: (4, 5, 6, 7)}
                job_i = 0
                for si in range(JCH):
                    jc = si
                    pool = simA if si % 2 == 0 else simB
                    g = pool.tile([P, 2, F], F32, tag="AB"[si % 2], name=f"g{ic}{pr}{si}")
                    for hb in range(2):
                        nc.tensor.matmul(
                            g[:, hb, :],
                            k_sb[hb * DH : (hb + 1) * DH, pr, jc * P : (jc + 1) * P],
                            q_sb[hb * DH : (hb + 1) * DH, pr, ic * F : (ic + 1) * F],
                            start=True,
                            stop=True,
                        )
                    nc.scalar.activation(
                        out=es[:, jc, :, :],
                        in_=g[:, :, :],
                        func=mybir.ActivationFunctionType.Exp,
                    )
                    if prev is not None and si == 0:
                        emit_den_chunk(pes, den, range(JCH))
                    if si in att_chunks:
                        emit_att_chunk(pes, ppr, att, att_chunks[si])
                    if prev is not None and si == 1:
                        nc.vector.reciprocal(out=den_sb[:, :, :], in_=den[:, :, :])
                    if last:
                        if si == 2:
                            # prev's chunks are all in, drain prev now so the
                            # att/mx slots can take this iteration's chase
                            emit_normalize(pic, ppr, att, den_sb)
                            satt = [
                                attp.tile([P, F], F32, tag="att", name="patsA"),
                                mxp.tile([P, F], F32, tag="mx", name="patsB"),
                            ]
                            # pairs 0-2 of every sub are already normalized:
                            # transpose them under this iteration's sim/exp
                            for sub in range(SUB):
                                nc.sync.dma_start_transpose(
                                    oallT[1][:, 0:3, sub * P : (sub + 1) * P],
                                    norm_sb[1][:, sub, 0 : 3 * P],
                                )
                        if si >= 2:
                            # old-orientation attn@v for the last pair:
                            # stationary [v | 1], moving exp'd sim
                            for hb in range(2):
                                nc.tensor.matmul(
                                    satt[hb][0 : DH + 1, :],
                                    vt1_sb[:, si - 2, hb, :],
                                    es[:, si - 2, hb, :],
                                    start=(si == 2),
                                    stop=False,
                                )
                    if si >= job_first_slot and job_i < len(proj_jobs):
                        proj_jobs[job_i]()
                        job_i += 1
                if prev is not None and not last:
                    emit_normalize(pic, ppr, att, den_sb)
                    if ppr == CCH - 1:
                        for sub in range(SUB):
                            emit_transpose(pic, sub)
                while job_i < len(proj_jobs):
                    proj_jobs[job_i]()
                    job_i += 1
                prev = (ic, pr, es)

            # ---- tail: finish the last pair's attn@v, normalize it straight
            # into oallT (it is already [c, i]-oriented), then o-proj ----
            pic, ppr, pes = prev
            for jc in (6, 7):
                for hb in range(2):
                    nc.tensor.matmul(
                        satt[hb][0 : DH + 1, :],
                        vt1_sb[:, jc, hb, :],
                        pes[:, jc, hb, :],
                        start=False,
                        stop=(jc == JCH - 1),
                    )
            for hb in range(2):
                den1 = sbcp.tile([1, F], F32, tag="d1", name=f"d1{hb}")
                nc.vector.reciprocal(out=den1[:, :], in_=satt[hb][DH : DH + 1, :])
                sdb = sbcp.tile([DH, F], F32, tag="sdb", name=f"sdb{hb}")
                nc.gpsimd.partition_broadcast(sdb[:, :], den1[:, :])
                nc.vector.tensor_tensor(
                    oallT[1][hb * DH : (hb + 1) * DH, 3, :],
                    satt[hb][0:DH, :],
                    sdb[:, :],
                    mybir.AluOpType.mult,
                )
            # keep-warm fillers: harmless matmuls into the drained jp slot
            # hold the PE p-state while the epilogue drains
            fb = jp.tile([P, F], F32, tag="jp", name="fillb")
            for w in range(5):
                nc.tensor.matmul(
                    fb[:, :],
                    pes[:, w % JCH, 0, 0:P],
                    vt_sb[:, w % JCH, :, :].rearrange("p h d -> p (h d)"),
                    start=True,
                    stop=True,
                )
            # the sim/jp psum slots are free in the tail: give each oproj
            # group its own slot so the four sub-chains run in parallel
            tail_slots = [(simA, "A"), (simB, "B"), (jp, "jp"), (attp, "att")]
            for sub in range(SUB):
                pool, tag = tail_slots[sub]
                emit_oproj_sub(1, sub, pool, tag)

    nc.compile()
    return nc


def prep_inputs(x, context, Wq, bq, Wk, bk, Wv, bv, Wo, bo):
    """Host-side sharding + layout prep. Returns per-core input maps."""
    xb = np.asarray(x, np.float32).reshape(B, C, NTOK).astype(NPBF16)
    cb = np.asarray(context, np.float32).reshape(B, C, NTOK).astype(NPBF16)
    wqt = np.ascontiguousarray((np.asarray(Wq, np.float32) * SCALE).T).astype(NPBF16)
    wkt = np.ascontiguousarray(np.asarray(Wk, np.float32).T).astype(NPBF16)
    wvt = np.ascontiguousarray(np.asarray(Wv, np.float32).T).astype(NPBF16)
    wot = np.ascontiguousarray(np.asarray(Wo, np.float32).T).astype(NPBF16)
    bqs = (np.asarray(bq, np.float32) * SCALE).astype(np.float32)
    bkf = np.asarray(bk, np.float32)
    bvf = np.asarray(bv, np.float32)
    bof = np.asarray(bo, np.float32)
    in_maps = []
    for b in range(B):
        in_maps.append(
            {
                "x": np.ascontiguousarray(xb[b]),
                "ctx": np.ascontiguousarray(cb[b]),
                "wqt": wqt,
                "wkt": wkt,
                "wvt": wvt,
                "wot": wot,
                "bq": bqs,
                "bk": bkf,
                "bv": bvf,
                "bo": bof,
            }
        )
    return in_maps


_NC = None


def _get_nc():
    global _NC
    if _NC is None:
        _NC = build_nc()
    return _NC


def kernel(x, context, Wq, bq, Wk, bk, Wv, bv, Wo, bo):
    from concourse.bass_utils import run_bass_kernel_spmd

    nc = _get_nc()
    in_maps = prep_inputs(x, context, Wq, bq, Wk, bk, Wv, bv, Wo, bo)
    br = run_bass_kernel_spmd(nc, in_maps, list(range(B)))
    out = np.stack([np.asarray(br.results[b]["out"], np.float32) for b in range(B)])
    return out.reshape(B, C, 32, 32)
